# revision 4
# baseline (speedup 1.0000x reference)
"""Trainium2 Bass kernel for a 4-layer MLP over N=100000 rows (DHGCN forward).

Reference computation (the graph edge_index `g` is dead):
    h = relu(x @ W0 + b0); h = relu(h @ W1 + b1)
    h = relu(h @ W2 + b2); h = relu(h @ W3 + b3)
with x [100000, 3000], W0 [3000,512], W1/W2 [512,512], W3 [512,20].

Strategy: data-parallel over rows across 8 NeuronCores (weights replicated).
On host, x is transposed to feature-major (xT), cast to bf16, and the feature
dim padded 3000 -> 3072 = 24*128 so activations live on-chip as
[feat_part, row] tiles; every matmul is then
out[M=out_feat_chunk, N=rows] = W_tile.T @ hT_tile with natural-layout
weights and no on-device transposes.

Matmul operands are bf16: on real TRN2 silicon fp32/f32r matmuls stream at
1/4 PE rate (measured: the f32r version of this kernel runs ~2.9 ms/core =
4 cycles/row, despite the cost model claiming f32r is full-rate at free
dim >= 256), while bf16 streams 1 row/cycle. PSUM accumulation stays fp32,
the final output is fp32; biases ride in bf16 (|b| <= 0.018 against
pre-activation std ~0.58, so bf16 bias rounding is ~7e-5 absolute — noise
next to the bf16 matmul chain's 5.1e-3) and are widened to fp32 on-chip
once at startup. Measured rel err of the full 4-layer chain is 5.1e-3,
comfortably inside the 2e-2 gate. fp8 (e4m3, 2x rate via DoubleRow) was
evaluated and rejected: 4.1e-2 rel err, over the gate.

The per-layer work is software-pipelined one row-block deep
(L0(j), L1(j-1), L2(j-2), L3(j-3)) so the PE never waits on the scalar
engine's relu/bias drain of the previous layer: every matmul's input tile
was produced a full block-iteration (~27 us) earlier. x arrives through a
block-contiguous DRAM layout (one 24 KB run per partition per block; the
original feature-major layout cost 24 strided 1 KB runs per partition) and
the w0/x0 startup loads are interleaved per K-tile so the first matmul
group starts as soon as tile 0 lands.

All weights and biases are packed into ONE DRAM tensor `wp` [128, 16477]
bf16 (w0 | w1 | w2 | w3 | b0 | b1 | b2 | b3 column-ranges, K-tiles
partition-major), so the kernel has exactly two input buffers (xb, wp).
Per-exec buffer-handle processing through the axon PJRT tunnel was measured
at ~60-80 us/handle (bench_dispatch.py: 1 vs 11 inputs), so dropping 7
handles trims the per-exec dispatch wall; on-device cost is unchanged
(same SBUF tiles, same matmul schedule, plus 4 one-time bf16->f32 bias
widening ops on the otherwise-idle ACT engine at startup).

Steady-state per-pass device time (measured differentially as
(wall(nrep=10)-wall(nrep=5))/5 with device-resident inputs, which cancels
the volatile 2-6 ms axon per-exec dispatch overhead): ~790-840 us/core in
the machine's fast state, drifting to ~1.5 ms/core under sustained load
(DVFS/thermal throttle — consistent with a 2.4->1.2 GHz PE step; the same
binary, same data re-measured minutes apart). The PE streaming floor is
687.5 us (3300 matmuls x 500 cols / 2.4 GHz); TimelineSim predicts 725.5 us
but costs InstLdweights at zero, and the per-matmul weight-reload is real
on silicon (~30 ns/matmul visible). Controls: a pe_tiny variant (identical
35k-instruction stream, 8-col matmuls) slopes at ~15 us/pass and dma_only
at ~113 us/pass, so the slope is genuine PE streaming time, not instruction
feed, sync, or DMA. Col-packing layer 3's M=20 matmuls via
tile_position=(0,32c) was tried and measured no faster.
"""

import numpy as np
from ml_dtypes import bfloat16

import concourse.bacc as bacc
import concourse.mybir as mybir
import concourse.tile as tile
from concourse.bass import ts
from concourse.bass_utils import run_bass_kernel_spmd

F32 = mybir.dt.float32
BF16 = mybir.dt.bfloat16
RELU = mybir.ActivationFunctionType.Relu
COPY = mybir.ActivationFunctionType.Copy

N_CORES = 8
N_ROWS = 100000
ROWS_PER_CORE = N_ROWS // N_CORES  # 12500
R = 500                            # row-block (PSUM free dim <= 512)
N_BLK = ROWS_PER_CORE // R         # 25
IN_DIM = 3000
K0 = 3072                          # padded in_dim = 24*128
KT0 = K0 // 128                    # 24 K-tiles for layer 0
H = 512
KT = H // 128                      # 4 K-tiles for layers 1-3
M_CH = H // 128                    # 4 output chunks of 128 for layers 0-2
LAT = 20

# column offsets inside the packed weight tensor wp [128, WCOLS] (bf16)
C_W0 = 0                           # [128, KT0*H]  = 12288
C_W1 = C_W0 + KT0 * H              # [128, KT*H]   = 2048
C_W2 = C_W1 + KT * H               # [128, KT*H]   = 2048
C_W3 = C_W2 + KT * H               # [128, KT*LAT] = 80
C_B0 = C_W3 + KT * LAT             # [128, M_CH]   = 4
C_B1 = C_B0 + M_CH
C_B2 = C_B1 + M_CH
C_B3 = C_B2 + M_CH                 # [LAT, 1]      = 1 (partitions 0..19)
WCOLS = C_B3 + 1


def build_program(nrep=1, psa_bufs=6, xin_bufs=3, hbuf_bufs=2):
    # psa_bufs=6: use 6 of the 8 PSUM banks for the L0-L2 accumulation
    # rotation (psB takes the rest). A/B-benched vs 4 banks with
    # time-interleaved batches: never slower, up to ~48 us/pass faster
    # (more WAR slack against scalar-engine PSUM drains), identical output.
    # nrep>1 repeats the whole block pipeline on-device (same data) —
    # used only for differential device-time measurement, since the axon
    # tunnel's per-exec dispatch overhead (volatile 2-6 ms) hides true HW
    # time.
    nc = bacc.Bacc("TRN2", target_bir_lowering=False, debug=False)

    # Block-contiguous layout: per row-block j, partition p holds its KT0*R
    # bf16 elements contiguously -> the per-block DMA is 128 runs of 24 KB
    # instead of 128*24 strided runs of 1 KB (24x fewer descriptors).
    xb = nc.dram_tensor("xb", [N_BLK, 128, KT0, R], BF16, kind="ExternalInput")
    wp = nc.dram_tensor("wp", [128, WCOLS], BF16, kind="ExternalInput")
    outT = nc.dram_tensor("outT", [LAT, ROWS_PER_CORE], F32, kind="ExternalOutput")

    with tile.TileContext(nc) as tc:
        with (
            tc.tile_pool(name="const", bufs=1) as const,
            tc.tile_pool(name="xin", bufs=xin_bufs) as xin,
            tc.tile_pool(name="hbuf", bufs=hbuf_bufs) as hbuf,
            tc.tile_pool(name="psA", bufs=psa_bufs, space="PSUM") as psA,
            tc.tile_pool(name="psB", bufs=2, space="PSUM") as psB,
        ):
            w0_sb = const.tile([128, KT0, H], BF16, tag="w0")
            w1_sb = const.tile([128, KT, H], BF16, tag="w1")
            w2_sb = const.tile([128, KT, H], BF16, tag="w2")
            w3_sb = const.tile([128, KT, LAT], BF16, tag="w3")
            b0_bf = const.tile([128, M_CH], BF16, tag="b0bf")
            b1_bf = const.tile([128, M_CH], BF16, tag="b1bf")
            b2_bf = const.tile([128, M_CH], BF16, tag="b2bf")
            b3_bf = const.tile([LAT, 1], BF16, tag="b3bf")
            b0_sb = const.tile([128, M_CH], F32, tag="b0")
            b1_sb = const.tile([128, M_CH], F32, tag="b1")
            b2_sb = const.tile([128, M_CH], F32, tag="b2")
            b3_sb = const.tile([LAT, 1], F32, tag="b3")
            out_sb = const.tile([LAT, ROWS_PER_CORE], F32, tag="out")

            h1s = [None] * N_BLK
            h2s = [None] * N_BLK
            h3s = [None] * N_BLK
            xts = [None] * N_BLK

            def load_w(w_sb, base, kt, cols):
                # per-K-tile DMA slices out of the packed tensor: each is a
                # contiguous cols*2-byte run per partition
                for ko in range(kt):
                    nc.sync.dma_start(
                        w_sb[:, ko, :], wp[:, base + ko * cols : base + (ko + 1) * cols]
                    )

            def load_bias(b_bf, b_f32, base, parts=128, m=M_CH):
                nc.sync.dma_start(b_bf[:], wp[:parts, base : base + m])
                nc.scalar.activation(b_f32[:], b_bf[:], COPY)

            def stage_l0(j, preload=-1):
                x_t = xin.tile([128, KT0, R], BF16, tag="x")
                xts[j] = x_t
                if preload == 0:
                    # w0/x0 interleaved per K-tile so the first matmul group
                    # starts once tile 0 lands; b0 (first needed by the
                    # first ACT, ~5 us later) rides behind the first pair
                    for ko in range(KT0):
                        nc.sync.dma_start(
                            w0_sb[:, ko, :],
                            wp[:, C_W0 + ko * H : C_W0 + (ko + 1) * H],
                        )
                        nc.sync.dma_start(x_t[:, ko, :], xb[j, :, ko, :])
                        if ko == 0:
                            load_bias(b0_bf, b0_sb, C_B0)
                else:
                    nc.sync.dma_start(x_t[:], xb[j])
                # Stagger the later-layer weight loads behind the first three
                # x-block DMAs: w_l is only needed once pipeline stage l
                # starts, and front-loading all of them delays x(1)/x(2).
                if preload == 1:
                    # defer each w_l to the iteration before stage l first
                    # needs it, so x(1)/x(2) aren't queued behind them
                    load_w(w1_sb, C_W1, KT, H)
                    load_bias(b1_bf, b1_sb, C_B1)
                elif preload == 2:
                    load_w(w2_sb, C_W2, KT, H)
                    load_bias(b2_bf, b2_sb, C_B2)
                elif preload == 3:
                    load_w(w3_sb, C_W3, KT, LAT)
                    load_bias(b3_bf, b3_sb, C_B3, parts=LAT, m=1)
                h1 = hbuf.tile([128, KT, R], BF16, tag="h1")
                h1s[j] = h1
                for m in range(M_CH):
                    ps = psA.tile([128, R], F32, tag="ps")
                    for ko in range(KT0):
                        nc.tensor.matmul(
                            ps[:],
                            w0_sb[:, ko, ts(m, 128)],
                            x_t[:, ko, :],
                            start=(ko == 0),
                            stop=(ko == KT0 - 1),
                        )
                    nc.scalar.activation(
                        h1[:, m, :], ps[:], RELU, bias=b0_sb[:, m : m + 1]
                    )

            def stage_mid(j, w_sb, b_sb, h_in, out_list, tag):
                h = hbuf.tile([128, KT, R], BF16, tag=tag)
                out_list[j] = h
                for m in range(M_CH):
                    ps = psA.tile([128, R], F32, tag="ps")
                    for ko in range(KT):
                        nc.tensor.matmul(
                            ps[:],
                            w_sb[:, ko, ts(m, 128)],
                            h_in[:, ko, :],
                            start=(ko == 0),
                            stop=(ko == KT - 1),
                        )
                    nc.scalar.activation(
                        h[:, m, :], ps[:], RELU, bias=b_sb[:, m : m + 1]
                    )

            def stage_l3(j):
                ps3 = psB.tile([LAT, R], F32, tag="ps3")
                for ko in range(KT):
                    nc.tensor.matmul(
                        ps3[:],
                        w3_sb[:, ko, :],
                        h3s[j][:, ko, :],
                        start=(ko == 0),
                        stop=(ko == KT - 1),
                    )
                nc.scalar.activation(
                    out_sb[:, ts(j, R)], ps3[:], RELU, bias=b3_sb[:]
                )

            for _rep in range(nrep):
                for j in range(N_BLK + 3):
                    if j < N_BLK:
                        stage_l0(j, preload=(j if _rep == 0 and j <= 3 else -1))
                    if 0 <= j - 1 < N_BLK:
                        stage_mid(j - 1, w1_sb, b1_sb, h1s[j - 1], h2s, "h2")
                    if 0 <= j - 2 < N_BLK:
                        stage_mid(j - 2, w2_sb, b2_sb, h2s[j - 2], h3s, "h3")
                    if 0 <= j - 3 < N_BLK:
                        stage_l3(j - 3)
                        # drain the first half of the output early so the
                        # final DMA tail only covers the second half
                        if _rep == nrep - 1 and j - 3 == N_BLK // 2:
                            nc.sync.dma_start(
                                outT[:, : (N_BLK // 2 + 1) * R],
                                out_sb[:, : (N_BLK // 2 + 1) * R],
                            )

            nc.sync.dma_start(
                outT[:, (N_BLK // 2 + 1) * R :], out_sb[:, (N_BLK // 2 + 1) * R :]
            )

    nc.compile()
    return nc


_NC = None


def _get_nc():
    global _NC
    if _NC is None:
        _NC = build_program()
    return _NC


def make_in_maps(inputs, W0, b0, W1, b1, W2, b2, W3, b3):
    """Host-side sharding: transpose x to feature-major bf16 in the
    block-contiguous device layout [N_BLK, 128, KT0, R] (features padded
    3000 -> 3072 with zeros in K-tile 23), rows sliced across cores;
    weights+biases replicated in one packed [128, WCOLS] bf16 tensor
    (partition-major K-tiles). Each core's "xb" is a contiguous view of
    one shared buffer so bass2jax's per-core np.asarray is copy-free.
    """
    x = np.asarray(inputs, dtype=np.float32)
    KF = IN_DIM // 128  # 23 full K-tiles; the remainder lands in tile 23
    xb_cat = np.zeros((N_CORES, N_BLK, 128, KT0, R), dtype=bfloat16)
    for c in range(N_CORES):
        xc = x[c * ROWS_PER_CORE : (c + 1) * ROWS_PER_CORE]
        bt = xc.reshape(N_BLK, R, IN_DIM)
        main = bt[..., : KF * 128].reshape(N_BLK, R, KF, 128)
        xb_cat[c, :, :, :KF, :] = main.transpose(0, 3, 2, 1).astype(bfloat16)
        xb_cat[c, :, : IN_DIM - KF * 128, KF, :] = (
            bt[..., KF * 128 :].transpose(0, 2, 1).astype(bfloat16)
        )

    wp = np.zeros((128, WCOLS), dtype=bfloat16)

    def put_w(W, kt, base):
        # [kt*128, F] -> [128, kt*F] (partition-major K-tiles)
        F = W.shape[1]
        Wp = np.zeros((kt * 128, F), dtype=np.float32)
        Wp[: W.shape[0]] = np.asarray(W, dtype=np.float32)
        wp[:, base : base + kt * F] = (
            Wp.reshape(kt, 128, F).transpose(1, 0, 2).reshape(128, kt * F)
            .astype(bfloat16)
        )

    put_w(np.asarray(W0), KT0, C_W0)
    put_w(np.asarray(W1), KT, C_W1)
    put_w(np.asarray(W2), KT, C_W2)
    put_w(np.asarray(W3), KT, C_W3)
    # biases: b[m*128 + p] -> wp[p, C_B + m]
    wp[:, C_B0 : C_B0 + M_CH] = (
        np.asarray(b0, np.float32).reshape(M_CH, 128).T.astype(bfloat16)
    )
    wp[:, C_B1 : C_B1 + M_CH] = (
        np.asarray(b1, np.float32).reshape(M_CH, 128).T.astype(bfloat16)
    )
    wp[:, C_B2 : C_B2 + M_CH] = (
        np.asarray(b2, np.float32).reshape(M_CH, 128).T.astype(bfloat16)
    )
    wp[:LAT, C_B3] = np.asarray(b3, np.float32).astype(bfloat16)

    in_maps = []
    for c in range(N_CORES):
        in_maps.append({"xb": xb_cat[c], "wp": wp})
    return in_maps


def kernel(inputs, g, W0, b0, W1, b1, W2, b2, W3, b3):
    nc = _get_nc()
    in_maps = make_in_maps(inputs, W0, b0, W1, b1, W2, b2, W3, b3)
    res = run_bass_kernel_spmd(nc, in_maps, core_ids=list(range(N_CORES)))
    out = np.empty((N_ROWS, LAT), dtype=np.float32)
    for c, r in enumerate(res.results):
        out[c * ROWS_PER_CORE : (c + 1) * ROWS_PER_CORE] = r["outT"].T
    return out


# revision 7
# speedup vs baseline: 1.0078x; 1.0078x over previous
"""Trainium2 Bass kernel for a 4-layer MLP over N=100000 rows (DHGCN forward).

Reference computation (the graph edge_index `g` is dead):
    h = relu(x @ W0 + b0); h = relu(h @ W1 + b1)
    h = relu(h @ W2 + b2); h = relu(h @ W3 + b3)
with x [100000, 3000], W0 [3000,512], W1/W2 [512,512], W3 [512,20].

Strategy: data-parallel over rows across 8 NeuronCores (weights replicated).
On host, x is transposed to feature-major (xT), cast to bf16, and the feature
dim padded 3000 -> 3072 = 24*128 so activations live on-chip as
[feat_part, row] tiles; every matmul is then
out[M=out_feat_chunk, N=rows] = W_tile.T @ hT_tile with natural-layout
weights and no on-device transposes.

Matmul operands are bf16: on real TRN2 silicon fp32/f32r matmuls stream at
1/4 PE rate (measured: the f32r version of this kernel runs ~2.9 ms/core =
4 cycles/row, despite the cost model claiming f32r is full-rate at free
dim >= 256), while bf16 streams 1 row/cycle. PSUM accumulation stays fp32,
the final output is fp32; biases ride in bf16 (|b| <= 0.018 against
pre-activation std ~0.58, so bf16 bias rounding is ~7e-5 absolute — noise
next to the bf16 matmul chain's 5.1e-3) and are widened to fp32 on-chip
once at startup. Measured rel err of the full 4-layer chain is 5.1e-3,
comfortably inside the 2e-2 gate. fp8 (e4m3, 2x rate via DoubleRow) was
evaluated and rejected: 4.1e-2 rel err, over the gate.

The per-layer work is software-pipelined one row-block deep
(L0(j), L1(j-1), L2(j-2), L3(j-3)) so the PE never waits on the scalar
engine's relu/bias drain of the previous layer: every matmul's input tile
was produced a full block-iteration (~27 us) earlier. x arrives through a
block-contiguous DRAM layout (one 24 KB run per partition per block; the
original feature-major layout cost 24 strided 1 KB runs per partition) and
the w0/x0 startup loads are interleaved per K-tile so the first matmul
group starts as soon as tile 0 lands.

All weights and biases are packed into ONE DRAM tensor `wp` [128, 16477]
bf16 (w0 | w1 | w2 | w3 | b0 | b1 | b2 | b3 column-ranges, K-tiles
partition-major), so the kernel has exactly two input buffers (xb, wp).
Per-exec buffer-handle processing through the axon PJRT tunnel was measured
at ~60-80 us/handle (bench_dispatch.py: 1 vs 11 inputs), so dropping 7
handles trims the per-exec dispatch wall; on-device cost is unchanged
(same SBUF tiles, same matmul schedule, plus 4 one-time bf16->f32 bias
widening ops on the otherwise-idle ACT engine at startup).

Steady-state per-pass device time (measured differentially as
(wall(nrep=10)-wall(nrep=5))/5 with device-resident inputs, which cancels
the volatile 2-6 ms axon per-exec dispatch overhead): ~790-840 us/core in
the machine's fast state, drifting to ~1.5 ms/core under sustained load
(DVFS/thermal throttle — consistent with a 2.4->1.2 GHz PE step; the same
binary, same data re-measured minutes apart). The PE streaming floor is
687.5 us (3300 matmuls x 500 cols / 2.4 GHz); TimelineSim predicts 725.5 us
but costs InstLdweights at zero, and the per-matmul weight-reload is real
on silicon (~30 ns/matmul visible). Controls: a pe_tiny variant (identical
35k-instruction stream, 8-col matmuls) slopes at ~15 us/pass and dma_only
at ~113 us/pass, so the slope is genuine PE streaming time, not instruction
feed, sync, or DMA. Col-packing layer 3's M=20 matmuls via
tile_position=(0,32c) was tried and measured no faster.
"""

import numpy as np
from ml_dtypes import bfloat16

import concourse.bacc as bacc
import concourse.mybir as mybir
import concourse.tile as tile
from concourse.bass import ts
from concourse.bass_utils import run_bass_kernel_spmd

F32 = mybir.dt.float32
BF16 = mybir.dt.bfloat16
RELU = mybir.ActivationFunctionType.Relu
COPY = mybir.ActivationFunctionType.Copy

N_CORES = 8
N_ROWS = 100000
ROWS_PER_CORE = N_ROWS // N_CORES  # 12500
R = 500                            # row-block (PSUM free dim <= 512)
N_BLK = ROWS_PER_CORE // R         # 25
IN_DIM = 3000
K0 = 3072                          # padded in_dim = 24*128
KT0 = K0 // 128                    # 24 K-tiles for layer 0
H = 512
KT = H // 128                      # 4 K-tiles for layers 1-3
M_CH = H // 128                    # 4 output chunks of 128 for layers 0-2
LAT = 20

# column offsets inside the packed weight tensor wp [128, WCOLS] (bf16)
C_W0 = 0                           # [128, KT0*H]  = 12288
C_W1 = C_W0 + KT0 * H              # [128, KT*H]   = 2048
C_W2 = C_W1 + KT * H               # [128, KT*H]   = 2048
C_W3 = C_W2 + KT * H               # [128, KT*LAT] = 80
C_B0 = C_W3 + KT * LAT             # [128, M_CH]   = 4
C_B1 = C_B0 + M_CH
C_B2 = C_B1 + M_CH
C_B3 = C_B2 + M_CH                 # [LAT, 1]      = 1 (partitions 0..19)
WCOLS = C_B3 + 1


def build_program(nrep=1, psa_bufs=6, xin_bufs=3, hbuf_bufs=2, x_chunks=24):
    # x_chunks=24: the per-block x load is issued as 24 per-K-tile DMAs
    # instead of one 3 MB burst. A/B by interleaved min-walls slope:
    # 1331 us vs 1371 us per pass (throttled state) — the burst contends
    # with PE SBUF reads (slope(full)-slope(pe_only) measured 15-48 us),
    # and finer chunks smooth it. Instruction count is free (pe_tiny
    # control: the whole 35k-instruction stream feeds at ~15 us/pass).
    # psa_bufs=6: use 6 of the 8 PSUM banks for the L0-L2 accumulation
    # rotation (psB takes the rest). A/B-benched vs 4 banks with
    # time-interleaved batches: never slower, up to ~48 us/pass faster
    # (more WAR slack against scalar-engine PSUM drains), identical output.
    # nrep>1 repeats the whole block pipeline on-device (same data) —
    # used only for differential device-time measurement, since the axon
    # tunnel's per-exec dispatch overhead (volatile 2-6 ms) hides true HW
    # time.
    nc = bacc.Bacc("TRN2", target_bir_lowering=False, debug=False)

    # Block-contiguous layout: per row-block j, partition p holds its KT0*R
    # bf16 elements contiguously -> the per-block DMA is 128 runs of 24 KB
    # instead of 128*24 strided runs of 1 KB (24x fewer descriptors).
    xb = nc.dram_tensor("xb", [N_BLK, 128, KT0, R], BF16, kind="ExternalInput")
    wp = nc.dram_tensor("wp", [128, WCOLS], BF16, kind="ExternalInput")
    outT = nc.dram_tensor("outT", [LAT, ROWS_PER_CORE], F32, kind="ExternalOutput")

    with tile.TileContext(nc) as tc:
        with (
            tc.tile_pool(name="const", bufs=1) as const,
            tc.tile_pool(name="xin", bufs=xin_bufs) as xin,
            tc.tile_pool(name="hbuf", bufs=hbuf_bufs) as hbuf,
            tc.tile_pool(name="psA", bufs=psa_bufs, space="PSUM") as psA,
            tc.tile_pool(name="psB", bufs=2, space="PSUM") as psB,
        ):
            w0_sb = const.tile([128, KT0, H], BF16, tag="w0")
            w1_sb = const.tile([128, KT, H], BF16, tag="w1")
            w2_sb = const.tile([128, KT, H], BF16, tag="w2")
            w3_sb = const.tile([128, KT, LAT], BF16, tag="w3")
            b0_bf = const.tile([128, M_CH], BF16, tag="b0bf")
            b1_bf = const.tile([128, M_CH], BF16, tag="b1bf")
            b2_bf = const.tile([128, M_CH], BF16, tag="b2bf")
            b3_bf = const.tile([LAT, 1], BF16, tag="b3bf")
            b0_sb = const.tile([128, M_CH], F32, tag="b0")
            b1_sb = const.tile([128, M_CH], F32, tag="b1")
            b2_sb = const.tile([128, M_CH], F32, tag="b2")
            b3_sb = const.tile([LAT, 1], F32, tag="b3")
            out_sb = const.tile([LAT, ROWS_PER_CORE], F32, tag="out")

            h1s = [None] * N_BLK
            h2s = [None] * N_BLK
            h3s = [None] * N_BLK
            xts = [None] * N_BLK

            def load_w(w_sb, base, kt, cols):
                # per-K-tile DMA slices out of the packed tensor: each is a
                # contiguous cols*2-byte run per partition
                for ko in range(kt):
                    nc.sync.dma_start(
                        w_sb[:, ko, :], wp[:, base + ko * cols : base + (ko + 1) * cols]
                    )

            def load_bias(b_bf, b_f32, base, parts=128, m=M_CH):
                nc.sync.dma_start(b_bf[:], wp[:parts, base : base + m])
                nc.scalar.activation(b_f32[:], b_bf[:], COPY)

            def stage_l0(j, preload=-1):
                x_t = xin.tile([128, KT0, R], BF16, tag="x")
                xts[j] = x_t
                if preload == 0:
                    # w0/x0 interleaved per K-tile so the first matmul group
                    # starts once tile 0 lands; b0 (first needed by the
                    # first ACT, ~5 us later) rides behind the first pair
                    for ko in range(KT0):
                        nc.sync.dma_start(
                            w0_sb[:, ko, :],
                            wp[:, C_W0 + ko * H : C_W0 + (ko + 1) * H],
                        )
                        nc.sync.dma_start(x_t[:, ko, :], xb[j, :, ko, :])
                        if ko == 0:
                            load_bias(b0_bf, b0_sb, C_B0)
                elif x_chunks == 1:
                    nc.sync.dma_start(x_t[:], xb[j])
                else:
                    # split the 3 MB block DMA into x_chunks smaller ones
                    # (each still contiguous per partition) so the burst
                    # contends less with PE SBUF reads
                    g = KT0 // x_chunks
                    for ci in range(x_chunks):
                        nc.sync.dma_start(
                            x_t[:, ci * g : (ci + 1) * g, :],
                            xb[j, :, ci * g : (ci + 1) * g, :],
                        )
                # Stagger the later-layer weight loads behind the first three
                # x-block DMAs: w_l is only needed once pipeline stage l
                # starts, and front-loading all of them delays x(1)/x(2).
                if preload == 1:
                    # defer each w_l to the iteration before stage l first
                    # needs it, so x(1)/x(2) aren't queued behind them
                    load_w(w1_sb, C_W1, KT, H)
                    load_bias(b1_bf, b1_sb, C_B1)
                elif preload == 2:
                    load_w(w2_sb, C_W2, KT, H)
                    load_bias(b2_bf, b2_sb, C_B2)
                elif preload == 3:
                    load_w(w3_sb, C_W3, KT, LAT)
                    load_bias(b3_bf, b3_sb, C_B3, parts=LAT, m=1)
                h1 = hbuf.tile([128, KT, R], BF16, tag="h1")
                h1s[j] = h1
                for m in range(M_CH):
                    ps = psA.tile([128, R], F32, tag="ps")
                    for ko in range(KT0):
                        nc.tensor.matmul(
                            ps[:],
                            w0_sb[:, ko, ts(m, 128)],
                            x_t[:, ko, :],
                            start=(ko == 0),
                            stop=(ko == KT0 - 1),
                        )
                    nc.scalar.activation(
                        h1[:, m, :], ps[:], RELU, bias=b0_sb[:, m : m + 1]
                    )

            def stage_mid(j, w_sb, b_sb, h_in, out_list, tag):
                h = hbuf.tile([128, KT, R], BF16, tag=tag)
                out_list[j] = h
                for m in range(M_CH):
                    ps = psA.tile([128, R], F32, tag="ps")
                    for ko in range(KT):
                        nc.tensor.matmul(
                            ps[:],
                            w_sb[:, ko, ts(m, 128)],
                            h_in[:, ko, :],
                            start=(ko == 0),
                            stop=(ko == KT - 1),
                        )
                    nc.scalar.activation(
                        h[:, m, :], ps[:], RELU, bias=b_sb[:, m : m + 1]
                    )

            def stage_l3(j):
                ps3 = psB.tile([LAT, R], F32, tag="ps3")
                for ko in range(KT):
                    nc.tensor.matmul(
                        ps3[:],
                        w3_sb[:, ko, :],
                        h3s[j][:, ko, :],
                        start=(ko == 0),
                        stop=(ko == KT - 1),
                    )
                nc.scalar.activation(
                    out_sb[:, ts(j, R)], ps3[:], RELU, bias=b3_sb[:]
                )

            for _rep in range(nrep):
                for j in range(N_BLK + 3):
                    if j < N_BLK:
                        stage_l0(j, preload=(j if _rep == 0 and j <= 3 else -1))
                    if 0 <= j - 1 < N_BLK:
                        stage_mid(j - 1, w1_sb, b1_sb, h1s[j - 1], h2s, "h2")
                    if 0 <= j - 2 < N_BLK:
                        stage_mid(j - 2, w2_sb, b2_sb, h2s[j - 2], h3s, "h3")
                    if 0 <= j - 3 < N_BLK:
                        stage_l3(j - 3)
                        # drain the first half of the output early so the
                        # final DMA tail only covers the second half
                        if _rep == nrep - 1 and j - 3 == N_BLK // 2:
                            nc.sync.dma_start(
                                outT[:, : (N_BLK // 2 + 1) * R],
                                out_sb[:, : (N_BLK // 2 + 1) * R],
                            )

            nc.sync.dma_start(
                outT[:, (N_BLK // 2 + 1) * R :], out_sb[:, (N_BLK // 2 + 1) * R :]
            )

    nc.compile()
    return nc


_NC = None


def _get_nc():
    global _NC
    if _NC is None:
        _NC = build_program()
    return _NC


def make_in_maps(inputs, W0, b0, W1, b1, W2, b2, W3, b3):
    """Host-side sharding: transpose x to feature-major bf16 in the
    block-contiguous device layout [N_BLK, 128, KT0, R] (features padded
    3000 -> 3072 with zeros in K-tile 23), rows sliced across cores;
    weights+biases replicated in one packed [128, WCOLS] bf16 tensor
    (partition-major K-tiles). Each core's "xb" is a contiguous view of
    one shared buffer so bass2jax's per-core np.asarray is copy-free.
    """
    x = np.asarray(inputs, dtype=np.float32)
    KF = IN_DIM // 128  # 23 full K-tiles; the remainder lands in tile 23
    xb_cat = np.zeros((N_CORES, N_BLK, 128, KT0, R), dtype=bfloat16)
    for c in range(N_CORES):
        xc = x[c * ROWS_PER_CORE : (c + 1) * ROWS_PER_CORE]
        bt = xc.reshape(N_BLK, R, IN_DIM)
        main = bt[..., : KF * 128].reshape(N_BLK, R, KF, 128)
        xb_cat[c, :, :, :KF, :] = main.transpose(0, 3, 2, 1).astype(bfloat16)
        xb_cat[c, :, : IN_DIM - KF * 128, KF, :] = (
            bt[..., KF * 128 :].transpose(0, 2, 1).astype(bfloat16)
        )

    wp = np.zeros((128, WCOLS), dtype=bfloat16)

    def put_w(W, kt, base):
        # [kt*128, F] -> [128, kt*F] (partition-major K-tiles)
        F = W.shape[1]
        Wp = np.zeros((kt * 128, F), dtype=np.float32)
        Wp[: W.shape[0]] = np.asarray(W, dtype=np.float32)
        wp[:, base : base + kt * F] = (
            Wp.reshape(kt, 128, F).transpose(1, 0, 2).reshape(128, kt * F)
            .astype(bfloat16)
        )

    put_w(np.asarray(W0), KT0, C_W0)
    put_w(np.asarray(W1), KT, C_W1)
    put_w(np.asarray(W2), KT, C_W2)
    put_w(np.asarray(W3), KT, C_W3)
    # biases: b[m*128 + p] -> wp[p, C_B + m]
    wp[:, C_B0 : C_B0 + M_CH] = (
        np.asarray(b0, np.float32).reshape(M_CH, 128).T.astype(bfloat16)
    )
    wp[:, C_B1 : C_B1 + M_CH] = (
        np.asarray(b1, np.float32).reshape(M_CH, 128).T.astype(bfloat16)
    )
    wp[:, C_B2 : C_B2 + M_CH] = (
        np.asarray(b2, np.float32).reshape(M_CH, 128).T.astype(bfloat16)
    )
    wp[:LAT, C_B3] = np.asarray(b3, np.float32).astype(bfloat16)

    in_maps = []
    for c in range(N_CORES):
        in_maps.append({"xb": xb_cat[c], "wp": wp})
    return in_maps


def kernel(inputs, g, W0, b0, W1, b1, W2, b2, W3, b3):
    nc = _get_nc()
    in_maps = make_in_maps(inputs, W0, b0, W1, b1, W2, b2, W3, b3)
    res = run_bass_kernel_spmd(nc, in_maps, core_ids=list(range(N_CORES)))
    out = np.empty((N_ROWS, LAT), dtype=np.float32)
    for c, r in enumerate(res.results):
        out[c * ROWS_PER_CORE : (c + 1) * ROWS_PER_CORE] = r["outT"].T
    return out
